# revision 5
# baseline (speedup 1.0000x reference)
"""Trainium2 Bass kernel for CAPE self-attention (DebugAttnProcessor).

Model (B=1, T_OUT=8, L=512, D=512, H=8; S = T_OUT*L = 4096, hd = 64):
    x = hidden_states reshaped (S, D)
    q/k/v = x @ Wq/Wk/Wv;  CAPE: per-frame 4x4 matrix applied to 4-groups of q,k
    scores = (q_h @ k_h^T) / sqrt(hd)  per head;  probs = softmax(scores)
    o = probs @ v_h;  out = concat(o) @ Wo + bo + residual

Sharding: tensor-parallel over heads -- core h owns head h.  The CAPE
transform and 1/sqrt(hd) scale are folded into per-frame effective Wq/Wk on
the host.  Each core computes the full (S, D) partial of the output
projection for its head; the host sums the 8 partials and adds bias +
residual (standard TP all-reduce epilogue).

On-core dataflow (fp32r matmuls; f32 tiles bitcast to f32r -- same bits):
    Phase A per frame: DMA x^T chunk, project q/k (M=64 accumulating chains),
    duplicate onto partitions 64:128 via SBUF DMA for the row-tiled score
    pairs; v via PE transpose into v_aug [keys, kt, 65] whose 65th column is
    ones (produces softmax denominators during AV).
    Phase B per 512-query chunk: scores as row-tiled concurrent pairs
    (tile_position (0,0)/(64,0)) writing 3-ktile PSUM groups [128,3,512];
    ACT exp per group (fewer, larger activations); AV as ONE matmul per
    ktile (K=128 keys, M=65, N=512) accumulating into a single PSUM bank.
    No max-subtraction: scores lie well inside fp32 exp range.
    Normalization by the denominator commutes with the output projection and
    is applied on the host during the unshard reduce.
    Phase C: output projection from o^T tiles, PSUM rotated through the
    shared score pool (6 banks) so copies overlap matmuls.
"""

import sys

if "/opt/trn_rl_repo" not in sys.path:
    sys.path.insert(0, "/opt/trn_rl_repo")

import numpy as np

# Model dims (hardcoded per problem spec)
B, T_OUT, L, D, H = 1, 8, 512, 512, 8
S = T_OUT * L            # 4096 tokens
HD = D // H              # 64 head dim
NC = 8                   # cores
P = 128                  # partitions
NCHUNK = D // P          # 4 contraction chunks of 128
KT = S // P              # 32 key tiles of 128
QCH = S // 512           # 8 query chunks of 512 (== frames)
KT_PER_F = 512 // P      # 4 key tiles per frame
GK = 3                   # ktiles per exp group

_CACHE = {}


def _build(reps: int = 1):
    """Build the single-core Bass program (head-agnostic; data picks the head)."""
    import concourse.bacc as bacc
    import concourse.mybir as mybir
    import concourse.tile as tile

    f32 = mybir.dt.float32
    f32r = mybir.dt.float32r
    AF = mybir.ActivationFunctionType

    nc = bacc.Bacc(trn_type="TRN2", target_bir_lowering=False, debug=False)

    xt_d = nc.dram_tensor("xt", [NCHUNK, P, S], f32, kind="ExternalInput")
    wq_d = nc.dram_tensor("wq", [P, NCHUNK, T_OUT, HD], f32, kind="ExternalInput")
    wk_d = nc.dram_tensor("wk", [P, NCHUNK, T_OUT, HD], f32, kind="ExternalInput")
    wv_d = nc.dram_tensor("wv", [P, NCHUNK, HD], f32, kind="ExternalInput")
    wo_d = nc.dram_tensor("wo", [HD, D], f32, kind="ExternalInput")
    out_d = nc.dram_tensor("out", [S, D], f32, kind="ExternalOutput")
    den_d = nc.dram_tensor("den", [QCH, 512], f32, kind="ExternalOutput")

    # exp groups per query chunk: ktiles [3,3,...,3,2]
    groups = []
    kt0 = 0
    while kt0 < KT:
        groups.append(list(range(kt0, min(kt0 + GK, KT))))
        kt0 += GK

    with tile.TileContext(nc) as tc:
        with (
            tc.tile_pool(name="persist", bufs=1) as persist,
            tc.tile_pool(name="stage", bufs=2) as stage,
            tc.tile_pool(name="probs", bufs=3) as probs_pool,
            tc.tile_pool(name="outp", bufs=3) as outp,
            tc.tile_pool(name="sc", bufs=2, space="PSUM") as sc_pool,
            tc.tile_pool(name="oacc", bufs=2, space="PSUM") as oacc_pool,
        ):
            # ---- persistent SBUF ----
            qT = persist.tile([P, S], f32r)          # rows 0:64 qT, 64:128 dup
            kT = persist.tile([P, S], f32r)
            v_aug = persist.tile([P, KT, HD + 1], f32r)  # [keys, kt, hd+ones]
            oT = persist.tile([HD, S], f32r)         # unnormalized o^T
            wq_s = persist.tile([P, NCHUNK, T_OUT, HD], f32r)
            wk_s = persist.tile([P, NCHUNK, T_OUT, HD], f32r)
            wv_s = persist.tile([P, NCHUNK, HD], f32r)
            wo_s = persist.tile([HD, D], f32r)
            ident = persist.tile([P, P], f32)

            # ---- load + round weights (fp32r matmul needs rounded inputs) ----
            wq_f = stage.tile([P, NCHUNK, T_OUT, HD], f32, tag="wload")
            nc.sync.dma_start(wq_f[:], wq_d[:])
            nc.vector.tensor_copy(wq_s[:], wq_f[:])
            wk_f = stage.tile([P, NCHUNK, T_OUT, HD], f32, tag="wload")
            nc.sync.dma_start(wk_f[:], wk_d[:])
            nc.vector.tensor_copy(wk_s[:], wk_f[:])
            wv_f = stage.tile([P, NCHUNK, HD], f32, tag="wload2")
            nc.sync.dma_start(wv_f[:], wv_d[:])
            nc.vector.tensor_copy(wv_s[:], wv_f[:])
            wo_f = stage.tile([HD, D], f32, tag="wload2")
            nc.sync.dma_start(wo_f[:], wo_d[:])
            nc.vector.tensor_copy(wo_s[:], wo_f[:])

            from concourse.masks import make_identity
            make_identity(nc, ident[:])

            # ones column of v_aug
            ones_f = stage.tile([P, 1], f32, tag="ones")
            nc.vector.memset(ones_f[:], 1.0)
            nc.vector.tensor_copy(
                v_aug[:, :, HD : HD + 1], ones_f[:, None, :].to_broadcast([P, KT, 1])
            )

            for _rep in range(reps):
                # ---- phase A: projections, per frame ----
                for f in range(T_OUT):
                    sl = slice(f * 512, (f + 1) * 512)
                    xt_f = stage.tile([P, NCHUNK, 512], f32, tag="xt")
                    nc.sync.dma_start(xt_f[:], xt_d[:, :, sl].rearrange("c p n -> p c n"))
                    xt_r = stage.tile([P, NCHUNK, 512], f32r, tag="xtr")
                    nc.vector.tensor_copy(xt_r[:], xt_f[:])

                    for which, w_s, dstT in (("q", wq_s, qT), ("k", wk_s, kT)):
                        ps_full = sc_pool.tile([P, GK, 512], f32, tag="sc", name="sc")
                        ps = ps_full[0:HD, 0, :]
                        for c in range(NCHUNK):
                            nc.tensor.matmul(
                                ps[:], w_s[:, c, f, :], xt_r[:, c, :],
                                start=(c == 0), stop=(c == NCHUNK - 1),
                            )
                        nc.vector.tensor_copy(dstT[0:HD, sl], ps[:])
                        # duplicate onto partitions 64:128 for row-tiled use
                        nc.sync.dma_start(dstT[HD:P, sl], dstT[0:HD, sl])

                    # v^T then PE-transpose into v_aug[keys, kt, 0:HD]
                    ps_full = sc_pool.tile([P, GK, 512], f32, tag="sc", name="sc")
                    ps = ps_full[0:HD, 0, :]
                    for c in range(NCHUNK):
                        nc.tensor.matmul(
                            ps[:], wv_s[:, c, :], xt_r[:, c, :],
                            start=(c == 0), stop=(c == NCHUNK - 1),
                        )
                    vT_f = stage.tile([HD, 512], f32, tag="vT")
                    nc.vector.tensor_copy(vT_f[:], ps[:])
                    for t in range(KT_PER_F):
                        vt_full = sc_pool.tile([P, GK, 512], f32, tag="sc", name="sc")
                        vt_ps = vt_full[:, 0, 0:HD]
                        nc.tensor.transpose(
                            vt_ps[:], vT_f[:, t * P : (t + 1) * P], ident[0:HD, 0:HD]
                        )
                        nc.vector.tensor_copy(
                            v_aug[:, f * KT_PER_F + t, 0:HD], vt_ps[:]
                        )

                # ---- phase B: attention per query chunk ----
                for fq in range(QCH):
                    qsl = slice(fq * 512, (fq + 1) * 512)
                    oA = oacc_pool.tile([P, 512], f32, tag="oA", name="oA")
                    for grp in groups:
                        sc = sc_pool.tile([P, GK, 512], f32, tag="sc", name="sc")
                        for j, kt in enumerate(grp):
                            half = kt % 2
                            hs = slice(0, HD) if half == 0 else slice(HD, P)
                            nc.tensor.matmul(
                                sc[:, j, :], kT[hs, kt * P : (kt + 1) * P],
                                qT[hs, qsl], start=True, stop=True,
                                tile_position=(half * HD, 0),
                            )
                        ng = len(grp)
                        pt = probs_pool.tile([P, GK, 512], f32r, tag="pt")
                        nc.scalar.activation(
                            pt[:, 0:ng, :].rearrange("p a b -> p (a b)"),
                            sc[:, 0:ng, :].rearrange("p a b -> p (a b)"), AF.Exp,
                        )
                        for j, kt in enumerate(grp):
                            nc.tensor.matmul(
                                oA[0 : HD + 1], v_aug[:, kt, :], pt[:, j, :],
                                start=(kt == 0), stop=(kt == KT - 1),
                            )
                    # epilogue: oT column + denominators (row 64).
                    nc.vector.tensor_copy(oT[:, qsl], oA[0:HD, :])
                    den_sb = stage.tile([1, 512], f32, tag="den")
                    nc.vector.tensor_copy(den_sb[:], oA[HD : HD + 1, :])
                    nc.sync.dma_start(den_d[fq, :], den_sb[:])

                    # ---- phase C: output projection for this frame ----
                    for t in range(KT_PER_F):
                        tt = fq * KT_PER_F + t
                        op_full = sc_pool.tile([P, GK, 512], f32, tag="sc", name="sc")
                        op_ps = op_full[:, 0, :]
                        nc.tensor.matmul(
                            op_ps[:], oT[:, tt * P : (tt + 1) * P], wo_s[:],
                            start=True, stop=True,
                        )
                        ot_sb = outp.tile([P, D], f32, tag="out")
                        nc.vector.tensor_copy(ot_sb[:], op_ps[:])
                        nc.sync.dma_start(out_d[tt * P : (tt + 1) * P, :], ot_sb[:])

    nc.compile()
    return nc


def _prep_inputs(hidden_states, p_out, p_out_inv, Wq, Wk, Wv, Wo):
    """Host-side: fold CAPE + scale into per-frame weights, shard by head."""
    x = np.ascontiguousarray(hidden_states, dtype=np.float32).reshape(S, D)
    xt = np.ascontiguousarray(x.reshape(S, NCHUNK, P).transpose(1, 2, 0))

    # Wq_eff[f] = Wq @ B_f with B_f = blockdiag(P_f per 4-group); scale on q
    Wq4 = np.asarray(Wq, np.float32).reshape(D, D // 4, 4)
    Wk4 = np.asarray(Wk, np.float32).reshape(D, D // 4, 4)
    Pq = np.asarray(p_out_inv, np.float32)[0]   # (T_OUT, 4, 4)
    Pk = np.asarray(p_out, np.float32)[0]
    scale = 1.0 / np.sqrt(HD)
    wq_eff = np.einsum("djk,fkg->fdjg", Wq4, Pq).reshape(T_OUT, D, D) * scale
    wk_eff = np.einsum("djk,fkg->fdjg", Wk4, Pk).reshape(T_OUT, D, D)

    in_maps = []
    Wv = np.asarray(Wv, np.float32)
    Wo = np.asarray(Wo, np.float32)
    for h in range(NC):
        cs = slice(h * HD, (h + 1) * HD)
        wq_h = np.ascontiguousarray(
            wq_eff[:, :, cs].reshape(T_OUT, NCHUNK, P, HD).transpose(2, 1, 0, 3)
        )
        wk_h = np.ascontiguousarray(
            wk_eff[:, :, cs].reshape(T_OUT, NCHUNK, P, HD).transpose(2, 1, 0, 3)
        )
        wv_h = np.ascontiguousarray(Wv[:, cs].reshape(NCHUNK, P, HD).transpose(1, 0, 2))
        wo_h = np.ascontiguousarray(Wo[cs, :])
        in_maps.append({"xt": xt, "wq": wq_h, "wk": wk_h, "wv": wv_h, "wo": wo_h})
    return in_maps


def run_sharded(inputs, trace=False):
    """Compile (cached), run on 8 cores, return (partials, BassKernelResults)."""
    from concourse.bass_utils import run_bass_kernel_spmd

    if "nc" not in _CACHE:
        _CACHE["nc"] = _build()
    nc = _CACHE["nc"]
    in_maps = _prep_inputs(
        inputs["hidden_states"], inputs["p_out"], inputs["p_out_inv"],
        inputs["Wq"], inputs["Wk"], inputs["Wv"], inputs["Wo"],
    )
    res = run_bass_kernel_spmd(nc, in_maps, core_ids=list(range(NC)), trace=trace)
    partials = np.stack([r["out"] for r in res.results])  # (8, S, D)
    dens = np.stack([r["den"].reshape(S) for r in res.results])  # (8, S)
    partials /= dens[:, :, None]
    return partials, res


def kernel(hidden_states, p_out, p_out_inv, Wq, Wk, Wv, Wo, bo, num_heads=None,
           **_unused):
    inputs = dict(hidden_states=hidden_states, p_out=p_out, p_out_inv=p_out_inv,
                  Wq=Wq, Wk=Wk, Wv=Wv, Wo=Wo)
    partials, _ = run_sharded(inputs)
    out = partials.sum(0, dtype=np.float64).astype(np.float32)
    out += np.asarray(bo, np.float32)[None, :]
    out += np.asarray(hidden_states, np.float32).reshape(S, D)
    return out.reshape(B * T_OUT, L, D)


# revision 7
# speedup vs baseline: 1.4101x; 1.4101x over previous
"""Trainium2 Bass kernel for CAPE self-attention (DebugAttnProcessor).

Model (B=1, T_OUT=8, L=512, D=512, H=8; S = T_OUT*L = 4096, hd = 64):
    x = hidden_states reshaped (S, D)
    q/k/v = x @ Wq/Wk/Wv;  CAPE: per-frame 4x4 matrix applied to 4-groups of q,k
    scores = (q_h @ k_h^T) / sqrt(hd)  per head;  probs = softmax(scores)
    o = probs @ v_h;  out = concat(o) @ Wo + bo + residual

Sharding: tensor-parallel over heads -- core h owns head h.  The CAPE
transform and 1/sqrt(hd) scale are folded into per-frame effective Wq/Wk on
the host.  Each core computes the full (S, D) partial of the output
projection for its head; the host sums the 8 partials and adds bias +
residual (standard TP all-reduce epilogue).

On-core dataflow (fp32r matmuls; f32 tiles bitcast to f32r -- same bits):
    Phase A per frame: DMA x^T chunk, project q/k (M=64 accumulating chains),
    duplicate onto partitions 64:128 via SBUF DMA for the row-tiled score
    pairs; v via PE transpose into v_aug [keys, kt, 65] whose 65th column is
    ones (produces softmax denominators during AV).
    Phase B per 512-query chunk: scores as row-tiled concurrent pairs
    (tile_position (0,0)/(64,0)) writing 3-ktile PSUM groups [128,3,512];
    ACT exp per group (fewer, larger activations); AV as ONE matmul per
    ktile (K=128 keys, M=65, N=512) accumulating into a single PSUM bank.
    No max-subtraction: scores lie well inside fp32 exp range.
    Normalization by the denominator commutes with the output projection and
    is applied on the host during the unshard reduce.
    Phase C: output projection from o^T tiles, PSUM rotated through the
    shared score pool (6 banks) so copies overlap matmuls.
"""

import sys

if "/opt/trn_rl_repo" not in sys.path:
    sys.path.insert(0, "/opt/trn_rl_repo")

import numpy as np

# Model dims (hardcoded per problem spec)
B, T_OUT, L, D, H = 1, 8, 512, 512, 8
S = T_OUT * L            # 4096 tokens
HD = D // H              # 64 head dim
NC = 8                   # cores
P = 128                  # partitions
NCHUNK = D // P          # 4 contraction chunks of 128
KT = S // P              # 32 key tiles of 128
QCH = S // 512           # 8 query chunks of 512 (== frames)
KT_PER_F = 512 // P      # 4 key tiles per frame
GK = 3                   # ktiles per exp group

_CACHE = {}


def _build(reps: int = 1):
    """Build the single-core Bass program (head-agnostic; data picks the head)."""
    import concourse.bacc as bacc
    import concourse.mybir as mybir
    import concourse.tile as tile

    f32 = mybir.dt.float32
    f32r = mybir.dt.float32r
    AF = mybir.ActivationFunctionType

    nc = bacc.Bacc(trn_type="TRN2", target_bir_lowering=False, debug=False)

    xt_d = nc.dram_tensor("xt", [NCHUNK, P, S], f32, kind="ExternalInput")
    wq_d = nc.dram_tensor("wq", [P, NCHUNK, T_OUT, HD], f32, kind="ExternalInput")
    wk_d = nc.dram_tensor("wk", [P, NCHUNK, T_OUT, HD], f32, kind="ExternalInput")
    wv_d = nc.dram_tensor("wv", [P, NCHUNK, HD], f32, kind="ExternalInput")
    wo_d = nc.dram_tensor("wo", [HD, D], f32, kind="ExternalInput")
    out_d = nc.dram_tensor("out", [S, D], f32, kind="ExternalOutput")
    den_d = nc.dram_tensor("den", [QCH, 512], f32, kind="ExternalOutput")

    # exp groups per query chunk: ktiles [3,3,...,3,2]
    groups = []
    kt0 = 0
    while kt0 < KT:
        groups.append(list(range(kt0, min(kt0 + GK, KT))))
        kt0 += GK

    with tile.TileContext(nc) as tc:
        with (
            tc.tile_pool(name="persist", bufs=1) as persist,
            tc.tile_pool(name="stage", bufs=2) as stage,
            tc.tile_pool(name="probs", bufs=3) as probs_pool,
            tc.tile_pool(name="outp", bufs=3) as outp,
            tc.tile_pool(name="sc", bufs=2, space="PSUM") as sc_pool,
            tc.tile_pool(name="oacc", bufs=1, space="PSUM") as oacc_pool,
            tc.tile_pool(name="cps", bufs=1, space="PSUM") as c_pool,
        ):
            # ---- persistent SBUF ----
            qT = persist.tile([P, S], f32r)          # rows 0:64 qT, 64:128 dup
            kT = persist.tile([P, S], f32r)
            v_aug = persist.tile([P, KT, HD + 1], f32r)  # [keys, kt, hd+ones]
            oT = persist.tile([HD, S], f32r)         # unnormalized o^T
            wq_s = persist.tile([P, NCHUNK, T_OUT, HD], f32r)
            wk_s = persist.tile([P, NCHUNK, T_OUT, HD], f32r)
            wv_s = persist.tile([P, NCHUNK, HD], f32r)
            wo_s = persist.tile([HD, D], f32r)
            ident = persist.tile([P, P], f32)

            # ---- load + round weights (fp32r matmul needs rounded inputs) ----
            wq_f = stage.tile([P, NCHUNK, T_OUT, HD], f32, tag="wload")
            nc.sync.dma_start(wq_f[:], wq_d[:])
            nc.vector.tensor_copy(wq_s[:], wq_f[:])
            wk_f = stage.tile([P, NCHUNK, T_OUT, HD], f32, tag="wload")
            nc.sync.dma_start(wk_f[:], wk_d[:])
            nc.vector.tensor_copy(wk_s[:], wk_f[:])
            wv_f = stage.tile([P, NCHUNK, HD], f32, tag="wload2")
            nc.sync.dma_start(wv_f[:], wv_d[:])
            nc.vector.tensor_copy(wv_s[:], wv_f[:])
            wo_f = stage.tile([HD, D], f32, tag="wload2")
            nc.sync.dma_start(wo_f[:], wo_d[:])
            nc.vector.tensor_copy(wo_s[:], wo_f[:])

            from concourse.masks import make_identity
            make_identity(nc, ident[:])

            # ones column of v_aug
            ones_f = stage.tile([P, 1], f32, tag="ones")
            nc.vector.memset(ones_f[:], 1.0)
            nc.vector.tensor_copy(
                v_aug[:, :, HD : HD + 1], ones_f[:, None, :].to_broadcast([P, KT, 1])
            )

            for _rep in range(reps):
                # ---- phase A: projections, per frame ----
                for f in range(T_OUT):
                    sl = slice(f * 512, (f + 1) * 512)
                    xt_f = stage.tile([P, NCHUNK, 512], f32, tag="xt")
                    nc.sync.dma_start(xt_f[:], xt_d[:, :, sl].rearrange("c p n -> p c n"))
                    xt_r = stage.tile([P, NCHUNK, 512], f32r, tag="xtr")
                    nc.vector.tensor_copy(xt_r[:], xt_f[:])

                    for which, w_s, dstT in (("q", wq_s, qT), ("k", wk_s, kT)):
                        ps_full = sc_pool.tile([P, GK, 512], f32, tag="sc", name="sc")
                        ps = ps_full[0:HD, 0, :]
                        for c in range(NCHUNK):
                            nc.tensor.matmul(
                                ps[:], w_s[:, c, f, :], xt_r[:, c, :],
                                start=(c == 0), stop=(c == NCHUNK - 1),
                            )
                        nc.vector.tensor_copy(dstT[0:HD, sl], ps[:])
                        # duplicate onto partitions 64:128 for row-tiled use
                        nc.sync.dma_start(dstT[HD:P, sl], dstT[0:HD, sl])

                    # v^T then PE-transpose into v_aug[keys, kt, 0:HD]
                    ps_full = sc_pool.tile([P, GK, 512], f32, tag="sc", name="sc")
                    ps = ps_full[0:HD, 0, :]
                    for c in range(NCHUNK):
                        nc.tensor.matmul(
                            ps[:], wv_s[:, c, :], xt_r[:, c, :],
                            start=(c == 0), stop=(c == NCHUNK - 1),
                        )
                    vT_f = stage.tile([HD, 512], f32, tag="vT")
                    nc.vector.tensor_copy(vT_f[:], ps[:])
                    for t in range(KT_PER_F):
                        vt_full = sc_pool.tile([P, GK, 512], f32, tag="sc", name="sc")
                        vt_ps = vt_full[:, 0, 0:HD]
                        nc.tensor.transpose(
                            vt_ps[:], vT_f[:, t * P : (t + 1) * P], ident[0:HD, 0:HD]
                        )
                        nc.vector.tensor_copy(
                            v_aug[:, f * KT_PER_F + t, 0:HD], vt_ps[:]
                        )

                # ---- phase B: attention per query chunk (software-pipelined:
                # scores for group g+1 are emitted BEFORE AV of group g so the
                # ACT exp stream never starves; output projection of the
                # previous query chunk dribbles through its own PSUM bank) ----
                def emit_scores(qsl, grp):
                    sc = sc_pool.tile([P, GK, 512], f32, tag="sc", name="sc")
                    for j, kt in enumerate(grp):
                        half = kt % 2
                        hs = slice(0, HD) if half == 0 else slice(HD, P)
                        nc.tensor.matmul(
                            sc[:, j, :], kT[hs, kt * P : (kt + 1) * P],
                            qT[hs, qsl], start=True, stop=True,
                            tile_position=(half * HD, 0),
                        )
                    return sc

                def emit_phase_c(tt):
                    op_ps = c_pool.tile([P, 512], f32, tag="cps", name="cps")
                    nc.tensor.matmul(
                        op_ps[:], oT[:, tt * P : (tt + 1) * P], wo_s[:],
                        start=True, stop=True,
                    )
                    ot_sb = outp.tile([P, D], f32, tag="out")
                    nc.vector.tensor_copy(ot_sb[:], op_ps[:])
                    nc.sync.dma_start(out_d[tt * P : (tt + 1) * P, :], ot_sb[:])

                for fq in range(QCH):
                    qsl = slice(fq * 512, (fq + 1) * 512)
                    oA = oacc_pool.tile([P, 512], f32, tag="oA", name="oA")
                    sc = emit_scores(qsl, groups[0])
                    for gi, grp in enumerate(groups):
                        ng = len(grp)
                        pt = probs_pool.tile([P, GK, 512], f32r, tag="pt")
                        nc.scalar.activation(
                            pt[:, 0:ng, :].rearrange("p a b -> p (a b)"),
                            sc[:, 0:ng, :].rearrange("p a b -> p (a b)"), AF.Exp,
                        )
                        if gi + 1 < len(groups):
                            sc = emit_scores(qsl, groups[gi + 1])
                        if fq > 0 and gi < KT_PER_F:
                            emit_phase_c((fq - 1) * KT_PER_F + gi)
                        for j, kt in enumerate(grp):
                            nc.tensor.matmul(
                                oA[0 : HD + 1], v_aug[:, kt, :], pt[:, j, :],
                                start=(kt == 0), stop=(kt == KT - 1),
                            )
                    # epilogue: oT column + denominators (row 64).
                    nc.vector.tensor_copy(oT[:, qsl], oA[0:HD, :])
                    den_sb = stage.tile([1, 512], f32, tag="den")
                    nc.vector.tensor_copy(den_sb[:], oA[HD : HD + 1, :])
                    nc.sync.dma_start(den_d[fq, :], den_sb[:])

                # tail: output projection for the last query chunk
                for t in range(KT_PER_F):
                    emit_phase_c((QCH - 1) * KT_PER_F + t)

    nc.compile()
    return nc


def _prep_inputs(hidden_states, p_out, p_out_inv, Wq, Wk, Wv, Wo):
    """Host-side: fold CAPE + scale into per-frame weights, shard by head."""
    x = np.ascontiguousarray(hidden_states, dtype=np.float32).reshape(S, D)
    xt = np.ascontiguousarray(x.reshape(S, NCHUNK, P).transpose(1, 2, 0))

    # Wq_eff[f] = Wq @ B_f with B_f = blockdiag(P_f per 4-group); scale on q
    Wq4 = np.asarray(Wq, np.float32).reshape(D, D // 4, 4)
    Wk4 = np.asarray(Wk, np.float32).reshape(D, D // 4, 4)
    Pq = np.asarray(p_out_inv, np.float32)[0]   # (T_OUT, 4, 4)
    Pk = np.asarray(p_out, np.float32)[0]
    scale = 1.0 / np.sqrt(HD)
    wq_eff = np.einsum("djk,fkg->fdjg", Wq4, Pq).reshape(T_OUT, D, D) * scale
    wk_eff = np.einsum("djk,fkg->fdjg", Wk4, Pk).reshape(T_OUT, D, D)

    in_maps = []
    Wv = np.asarray(Wv, np.float32)
    Wo = np.asarray(Wo, np.float32)
    for h in range(NC):
        cs = slice(h * HD, (h + 1) * HD)
        wq_h = np.ascontiguousarray(
            wq_eff[:, :, cs].reshape(T_OUT, NCHUNK, P, HD).transpose(2, 1, 0, 3)
        )
        wk_h = np.ascontiguousarray(
            wk_eff[:, :, cs].reshape(T_OUT, NCHUNK, P, HD).transpose(2, 1, 0, 3)
        )
        wv_h = np.ascontiguousarray(Wv[:, cs].reshape(NCHUNK, P, HD).transpose(1, 0, 2))
        wo_h = np.ascontiguousarray(Wo[cs, :])
        in_maps.append({"xt": xt, "wq": wq_h, "wk": wk_h, "wv": wv_h, "wo": wo_h})
    return in_maps


def run_sharded(inputs, trace=False):
    """Compile (cached), run on 8 cores, return (partials, BassKernelResults)."""
    from concourse.bass_utils import run_bass_kernel_spmd

    if "nc" not in _CACHE:
        _CACHE["nc"] = _build()
    nc = _CACHE["nc"]
    in_maps = _prep_inputs(
        inputs["hidden_states"], inputs["p_out"], inputs["p_out_inv"],
        inputs["Wq"], inputs["Wk"], inputs["Wv"], inputs["Wo"],
    )
    res = run_bass_kernel_spmd(nc, in_maps, core_ids=list(range(NC)), trace=trace)
    partials = np.stack([r["out"] for r in res.results])  # (8, S, D)
    dens = np.stack([r["den"].reshape(S) for r in res.results])  # (8, S)
    partials /= dens[:, :, None]
    return partials, res


def kernel(hidden_states, p_out, p_out_inv, Wq, Wk, Wv, Wo, bo, num_heads=None,
           **_unused):
    inputs = dict(hidden_states=hidden_states, p_out=p_out, p_out_inv=p_out_inv,
                  Wq=Wq, Wk=Wk, Wv=Wv, Wo=Wo)
    partials, _ = run_sharded(inputs)
    out = partials.sum(0, dtype=np.float64).astype(np.float32)
    out += np.asarray(bo, np.float32)[None, :]
    out += np.asarray(hidden_states, np.float32).reshape(S, D)
    return out.reshape(B * T_OUT, L, D)


# revision 8
# speedup vs baseline: 1.5332x; 1.0873x over previous
"""Trainium2 Bass kernel for CAPE self-attention (DebugAttnProcessor).

Model (B=1, T_OUT=8, L=512, D=512, H=8; S = T_OUT*L = 4096, hd = 64):
    x = hidden_states reshaped (S, D)
    q/k/v = x @ Wq/Wk/Wv;  CAPE: per-frame 4x4 matrix applied to 4-groups of q,k
    scores = (q_h @ k_h^T) / sqrt(hd)  per head;  probs = softmax(scores)
    o = probs @ v_h;  out = concat(o) @ Wo + bo + residual

Sharding: tensor-parallel over heads -- core h owns head h.  The CAPE
transform and 1/sqrt(hd) scale are folded into per-frame effective Wq/Wk on
the host.  Each core computes the full (S, D) partial of the output
projection for its head; the host sums the 8 partials and adds bias +
residual (standard TP all-reduce epilogue).

On-core dataflow (fp32r matmuls; f32 tiles bitcast to f32r -- same bits):
    Phase A per frame: DMA x^T chunk, project q/k (M=64 accumulating chains),
    duplicate onto partitions 64:128 via SBUF DMA for the row-tiled score
    pairs; v via PE transpose into v_aug [keys, kt, 65] whose 65th column is
    ones (produces softmax denominators during AV).
    Phase B per 512-query chunk: scores as row-tiled concurrent pairs
    (tile_position (0,0)/(64,0)) writing 3-ktile PSUM groups [128,3,512];
    ACT exp per group (fewer, larger activations); AV as ONE matmul per
    ktile (K=128 keys, M=65, N=512) accumulating into a single PSUM bank.
    No max-subtraction: scores lie well inside fp32 exp range.
    Normalization by the denominator commutes with the output projection and
    is applied on the host during the unshard reduce.
    Phase C: output projection from o^T tiles, PSUM rotated through the
    shared score pool (6 banks) so copies overlap matmuls.
"""

import sys

if "/opt/trn_rl_repo" not in sys.path:
    sys.path.insert(0, "/opt/trn_rl_repo")

import numpy as np

# Model dims (hardcoded per problem spec)
B, T_OUT, L, D, H = 1, 8, 512, 512, 8
S = T_OUT * L            # 4096 tokens
HD = D // H              # 64 head dim
NC = 8                   # cores
P = 128                  # partitions
NCHUNK = D // P          # 4 contraction chunks of 128
KT = S // P              # 32 key tiles of 128
QCH = S // 512           # 8 query chunks of 512 (== frames)
KT_PER_F = 512 // P      # 4 key tiles per frame
GK = 2                   # ktiles per exp group

_CACHE = {}


def _build(reps: int = 1):
    """Build the single-core Bass program (head-agnostic; data picks the head)."""
    import concourse.bacc as bacc
    import concourse.mybir as mybir
    import concourse.tile as tile

    f32 = mybir.dt.float32
    f32r = mybir.dt.float32r
    AF = mybir.ActivationFunctionType

    nc = bacc.Bacc(trn_type="TRN2", target_bir_lowering=False, debug=False)

    xt_d = nc.dram_tensor("xt", [NCHUNK, P, S], f32, kind="ExternalInput")
    wq_d = nc.dram_tensor("wq", [P, NCHUNK, T_OUT, HD], f32, kind="ExternalInput")
    wk_d = nc.dram_tensor("wk", [P, NCHUNK, T_OUT, HD], f32, kind="ExternalInput")
    wv_d = nc.dram_tensor("wv", [P, NCHUNK, HD], f32, kind="ExternalInput")
    wo_d = nc.dram_tensor("wo", [HD, D], f32, kind="ExternalInput")
    out_d = nc.dram_tensor("out", [S, D], f32, kind="ExternalOutput")
    den_d = nc.dram_tensor("den", [QCH, 512], f32, kind="ExternalOutput")

    # exp groups per query chunk: ktiles [3,3,...,3,2]
    groups = []
    kt0 = 0
    while kt0 < KT:
        groups.append(list(range(kt0, min(kt0 + GK, KT))))
        kt0 += GK

    with tile.TileContext(nc) as tc:
        with (
            tc.tile_pool(name="persist", bufs=1) as persist,
            tc.tile_pool(name="stage", bufs=2) as stage,
            tc.tile_pool(name="probs", bufs=3) as probs_pool,
            tc.tile_pool(name="outp", bufs=3) as outp,
            tc.tile_pool(name="sc", bufs=3, space="PSUM") as sc_pool,
            tc.tile_pool(name="oacc", bufs=1, space="PSUM") as oacc_pool,
            tc.tile_pool(name="cps", bufs=1, space="PSUM") as c_pool,
        ):
            # ---- persistent SBUF ----
            qT = persist.tile([P, S], f32r)          # rows 0:64 qT, 64:128 dup
            kT = persist.tile([P, S], f32r)
            v_aug = persist.tile([P, KT, HD + 1], f32r)  # [keys, kt, hd+ones]
            oT = persist.tile([HD, S], f32r)         # unnormalized o^T
            wq_s = persist.tile([P, NCHUNK, T_OUT, HD], f32r)
            wk_s = persist.tile([P, NCHUNK, T_OUT, HD], f32r)
            wv_s = persist.tile([P, NCHUNK, HD], f32r)
            wo_s = persist.tile([HD, D], f32r)
            ident = persist.tile([P, P], f32)

            # ---- load + round weights (fp32r matmul needs rounded inputs) ----
            wq_f = stage.tile([P, NCHUNK, T_OUT, HD], f32, tag="wload")
            nc.sync.dma_start(wq_f[:], wq_d[:])
            nc.vector.tensor_copy(wq_s[:], wq_f[:])
            wk_f = stage.tile([P, NCHUNK, T_OUT, HD], f32, tag="wload")
            nc.sync.dma_start(wk_f[:], wk_d[:])
            nc.vector.tensor_copy(wk_s[:], wk_f[:])
            wv_f = stage.tile([P, NCHUNK, HD], f32, tag="wload2")
            nc.sync.dma_start(wv_f[:], wv_d[:])
            nc.vector.tensor_copy(wv_s[:], wv_f[:])
            wo_f = stage.tile([HD, D], f32, tag="wload2")
            nc.sync.dma_start(wo_f[:], wo_d[:])
            nc.vector.tensor_copy(wo_s[:], wo_f[:])

            from concourse.masks import make_identity
            make_identity(nc, ident[:])

            # ones column of v_aug
            ones_f = stage.tile([P, 1], f32, tag="ones")
            nc.vector.memset(ones_f[:], 1.0)
            nc.vector.tensor_copy(
                v_aug[:, :, HD : HD + 1], ones_f[:, None, :].to_broadcast([P, KT, 1])
            )

            for _rep in range(reps):
                # ---- phase A: projections, per frame ----
                for f in range(T_OUT):
                    sl = slice(f * 512, (f + 1) * 512)
                    xt_f = stage.tile([P, NCHUNK, 512], f32, tag="xt")
                    nc.sync.dma_start(xt_f[:], xt_d[:, :, sl].rearrange("c p n -> p c n"))
                    xt_r = stage.tile([P, NCHUNK, 512], f32r, tag="xtr")
                    nc.vector.tensor_copy(xt_r[:], xt_f[:])

                    for which, w_s, dstT in (("q", wq_s, qT), ("k", wk_s, kT)):
                        ps_full = sc_pool.tile([P, GK, 512], f32, tag="sc", name="sc")
                        ps = ps_full[0:HD, 0, :]
                        for c in range(NCHUNK):
                            nc.tensor.matmul(
                                ps[:], w_s[:, c, f, :], xt_r[:, c, :],
                                start=(c == 0), stop=(c == NCHUNK - 1),
                            )
                        nc.vector.tensor_copy(dstT[0:HD, sl], ps[:])
                        # duplicate onto partitions 64:128 for row-tiled use
                        nc.sync.dma_start(dstT[HD:P, sl], dstT[0:HD, sl])

                    # v^T then PE-transpose into v_aug[keys, kt, 0:HD]
                    ps_full = sc_pool.tile([P, GK, 512], f32, tag="sc", name="sc")
                    ps = ps_full[0:HD, 0, :]
                    for c in range(NCHUNK):
                        nc.tensor.matmul(
                            ps[:], wv_s[:, c, :], xt_r[:, c, :],
                            start=(c == 0), stop=(c == NCHUNK - 1),
                        )
                    vT_f = stage.tile([HD, 512], f32, tag="vT")
                    nc.vector.tensor_copy(vT_f[:], ps[:])
                    for t in range(KT_PER_F):
                        vt_full = sc_pool.tile([P, GK, 512], f32, tag="sc", name="sc")
                        vt_ps = vt_full[:, 0, 0:HD]
                        nc.tensor.transpose(
                            vt_ps[:], vT_f[:, t * P : (t + 1) * P], ident[0:HD, 0:HD]
                        )
                        nc.vector.tensor_copy(
                            v_aug[:, f * KT_PER_F + t, 0:HD], vt_ps[:]
                        )

                # ---- phase B: attention per query chunk (software-pipelined:
                # scores for group g+1 are emitted BEFORE AV of group g so the
                # ACT exp stream never starves; output projection of the
                # previous query chunk dribbles through its own PSUM bank) ----
                def emit_scores(qsl, grp):
                    sc = sc_pool.tile([P, GK, 512], f32, tag="sc", name="sc")
                    for j, kt in enumerate(grp):
                        half = kt % 2
                        hs = slice(0, HD) if half == 0 else slice(HD, P)
                        nc.tensor.matmul(
                            sc[:, j, :], kT[hs, kt * P : (kt + 1) * P],
                            qT[hs, qsl], start=True, stop=True,
                            tile_position=(half * HD, 0),
                        )
                    return sc

                def emit_phase_c(tt):
                    op_ps = c_pool.tile([P, 512], f32, tag="cps", name="cps")
                    nc.tensor.matmul(
                        op_ps[:], oT[:, tt * P : (tt + 1) * P], wo_s[:],
                        start=True, stop=True,
                    )
                    ot_sb = outp.tile([P, D], f32, tag="out")
                    nc.vector.tensor_copy(ot_sb[:], op_ps[:])
                    nc.sync.dma_start(out_d[tt * P : (tt + 1) * P, :], ot_sb[:])

                for fq in range(QCH):
                    qsl = slice(fq * 512, (fq + 1) * 512)
                    oA = oacc_pool.tile([P, 512], f32, tag="oA", name="oA")
                    sc_q = [emit_scores(qsl, groups[0]), emit_scores(qsl, groups[1])]
                    for gi, grp in enumerate(groups):
                        ng = len(grp)
                        sc = sc_q.pop(0)
                        pt = probs_pool.tile([P, GK, 512], f32r, tag="pt")
                        nc.scalar.activation(
                            pt[:, 0:ng, :].rearrange("p a b -> p (a b)"),
                            sc[:, 0:ng, :].rearrange("p a b -> p (a b)"), AF.Exp,
                        )
                        if gi + 2 < len(groups):
                            sc_q.append(emit_scores(qsl, groups[gi + 2]))
                        if fq > 0 and gi < KT_PER_F:
                            emit_phase_c((fq - 1) * KT_PER_F + gi)
                        for j, kt in enumerate(grp):
                            nc.tensor.matmul(
                                oA[0 : HD + 1], v_aug[:, kt, :], pt[:, j, :],
                                start=(kt == 0), stop=(kt == KT - 1),
                            )
                    # epilogue: oT column + denominators (row 64).
                    nc.vector.tensor_copy(oT[:, qsl], oA[0:HD, :])
                    den_sb = stage.tile([1, 512], f32, tag="den")
                    nc.vector.tensor_copy(den_sb[:], oA[HD : HD + 1, :])
                    nc.sync.dma_start(den_d[fq, :], den_sb[:])

                # tail: output projection for the last query chunk
                for t in range(KT_PER_F):
                    emit_phase_c((QCH - 1) * KT_PER_F + t)

    nc.compile()
    return nc


def _prep_inputs(hidden_states, p_out, p_out_inv, Wq, Wk, Wv, Wo):
    """Host-side: fold CAPE + scale into per-frame weights, shard by head."""
    x = np.ascontiguousarray(hidden_states, dtype=np.float32).reshape(S, D)
    xt = np.ascontiguousarray(x.reshape(S, NCHUNK, P).transpose(1, 2, 0))

    # Wq_eff[f] = Wq @ B_f with B_f = blockdiag(P_f per 4-group); scale on q
    Wq4 = np.asarray(Wq, np.float32).reshape(D, D // 4, 4)
    Wk4 = np.asarray(Wk, np.float32).reshape(D, D // 4, 4)
    Pq = np.asarray(p_out_inv, np.float32)[0]   # (T_OUT, 4, 4)
    Pk = np.asarray(p_out, np.float32)[0]
    scale = 1.0 / np.sqrt(HD)
    wq_eff = np.einsum("djk,fkg->fdjg", Wq4, Pq).reshape(T_OUT, D, D) * scale
    wk_eff = np.einsum("djk,fkg->fdjg", Wk4, Pk).reshape(T_OUT, D, D)

    in_maps = []
    Wv = np.asarray(Wv, np.float32)
    Wo = np.asarray(Wo, np.float32)
    for h in range(NC):
        cs = slice(h * HD, (h + 1) * HD)
        wq_h = np.ascontiguousarray(
            wq_eff[:, :, cs].reshape(T_OUT, NCHUNK, P, HD).transpose(2, 1, 0, 3)
        )
        wk_h = np.ascontiguousarray(
            wk_eff[:, :, cs].reshape(T_OUT, NCHUNK, P, HD).transpose(2, 1, 0, 3)
        )
        wv_h = np.ascontiguousarray(Wv[:, cs].reshape(NCHUNK, P, HD).transpose(1, 0, 2))
        wo_h = np.ascontiguousarray(Wo[cs, :])
        in_maps.append({"xt": xt, "wq": wq_h, "wk": wk_h, "wv": wv_h, "wo": wo_h})
    return in_maps


def run_sharded(inputs, trace=False):
    """Compile (cached), run on 8 cores, return (partials, BassKernelResults)."""
    from concourse.bass_utils import run_bass_kernel_spmd

    if "nc" not in _CACHE:
        _CACHE["nc"] = _build()
    nc = _CACHE["nc"]
    in_maps = _prep_inputs(
        inputs["hidden_states"], inputs["p_out"], inputs["p_out_inv"],
        inputs["Wq"], inputs["Wk"], inputs["Wv"], inputs["Wo"],
    )
    res = run_bass_kernel_spmd(nc, in_maps, core_ids=list(range(NC)), trace=trace)
    partials = np.stack([r["out"] for r in res.results])  # (8, S, D)
    dens = np.stack([r["den"].reshape(S) for r in res.results])  # (8, S)
    partials /= dens[:, :, None]
    return partials, res


def kernel(hidden_states, p_out, p_out_inv, Wq, Wk, Wv, Wo, bo, num_heads=None,
           **_unused):
    inputs = dict(hidden_states=hidden_states, p_out=p_out, p_out_inv=p_out_inv,
                  Wq=Wq, Wk=Wk, Wv=Wv, Wo=Wo)
    partials, _ = run_sharded(inputs)
    out = partials.sum(0, dtype=np.float64).astype(np.float32)
    out += np.asarray(bo, np.float32)[None, :]
    out += np.asarray(hidden_states, np.float32).reshape(S, D)
    return out.reshape(B * T_OUT, L, D)
